# revision 1
# baseline (speedup 1.0000x reference)
"""Margin-based triplet criterion (loss_fn) on 8 TRN2 NeuronCores.

Strategy (data-parallel over the triplet dim T, per the sharding hint):
  - Host: cast batch to bf16 (replicated to all cores), precompute per-row
    squared norms s[r] = sum(batch_bf16[r]**2) (fp32), per-triplet
    ssum_ap = s[ia]+s[ip], ssum_an = s[ia]+s[in], and the per-triplet hinge
    thresholds bm = beta[labels[ia]] - margin, bp = beta[labels[ia]] + margin.
    Shard triplets T=65536 -> 8192 per core.
  - Device (per core): indirect row gather (128 rows per SWDGE instruction,
    one row per partition) pulls a/p/n rows into [128, GJ, 512] bf16 tiles.
    DVE computes elementwise products a*p, a*n (bf16 2x mode) and reduces
    each 512-segment (free dim) -> dot products, laid out [128, cols].
    d^2 = ssum - 2*dot (clamped at 0), d = sqrt(d^2 + eps) on ACT, hinge
    losses + pair indicator + free-dim reductions on DVE -> [128, 2]
    partial (sum, count) per core.
  - Host: sum the 8x128 partials, loss = total / max(count, 1) if count > 0.

Triplet t of a core maps to (partition p, column f) with t = p*ROWS + f,
ROWS = 64. Gather instruction (class k, column f) uses idx[:, k*ROWS+f].
"""

import numpy as np
import ml_dtypes
from contextlib import ExitStack

import concourse.bass as bass
import concourse.bacc as bacc
import concourse.tile as tile
from concourse import mybir
from concourse.bass_utils import run_bass_kernel_spmd

N_CORES = 8
B, D, T, C = 4096, 512, 65536, 100
T_LOC = T // N_CORES            # 8192 triplets per core
ROWS = T_LOC // 128             # 64 gather groups / epilogue free dim
GJ = 16                         # gather groups buffered per product tile
N_CHUNKS = ROWS // GJ           # 4
MARGIN = 0.2
EPS = 1e-8

f32 = mybir.dt.float32
bf16 = mybir.dt.bfloat16
i32 = mybir.dt.int32

_CACHE = {}


def _build_nc():
    nc = bacc.Bacc(
        "TRN2", target_bir_lowering=False, debug=False,
        enable_asserts=False, num_devices=N_CORES,
    )
    bt = nc.dram_tensor("bt", [B, D], bf16, kind="ExternalInput")
    idx = nc.dram_tensor("idx", [128, 3 * ROWS], i32, kind="ExternalInput")
    ssum_ap = nc.dram_tensor("ssum_ap", [128, ROWS], f32, kind="ExternalInput")
    ssum_an = nc.dram_tensor("ssum_an", [128, ROWS], f32, kind="ExternalInput")
    bm = nc.dram_tensor("bm", [128, ROWS], f32, kind="ExternalInput")
    bp = nc.dram_tensor("bp", [128, ROWS], f32, kind="ExternalInput")
    outp = nc.dram_tensor("out", [128, 2], f32, kind="ExternalOutput")

    with tile.TileContext(nc) as tc, ExitStack() as ctx:
        const_pool = ctx.enter_context(tc.tile_pool(name="const", bufs=1))
        gath_pool = ctx.enter_context(tc.tile_pool(name="gath", bufs=2))
        epi_pool = ctx.enter_context(tc.tile_pool(name="epi", bufs=1))

        eps_sb = const_pool.tile([128, 1], f32)
        nc.vector.memset(eps_sb[:], EPS)
        idx_sb = const_pool.tile([128, 3 * ROWS], i32)
        nc.sync.dma_start(idx_sb[:], idx[:])
        scal_sb = {}
        for name, dram in (("ssum_ap", ssum_ap), ("ssum_an", ssum_an),
                           ("bm", bm), ("bp", bp)):
            t = const_pool.tile([128, ROWS], f32, tag=name, name=name + "_sb")
            nc.sync.dma_start(t[:], dram[:])
            scal_sb[name] = t
        dsq = {
            d: epi_pool.tile([128, ROWS], f32, tag=f"dsq_{d}", name=f"dsq_{d}")
            for d in ("ap", "an")
        }

        for ci in range(N_CHUNKS):
            g = {}
            for ki, k in enumerate(("a", "p", "n")):
                gt = gath_pool.tile([128, GJ, D], bf16, tag=f"g_{k}", name=f"g_{k}")
                for j in range(GJ):
                    f = ci * GJ + j
                    nc.gpsimd.indirect_dma_start(
                        out=gt[:, j, :],
                        out_offset=None,
                        in_=bt[:],
                        in_offset=bass.IndirectOffsetOnAxis(
                            ap=idx_sb[:, ki * ROWS + f: ki * ROWS + f + 1],
                            axis=0),
                    )
                g[k] = gt
            # products in place (p <- a*p, n <- a*n), then 512-segment reduce
            for d, other in (("ap", "p"), ("an", "n")):
                nc.vector.tensor_tensor(
                    out=g[other][:], in0=g["a"][:], in1=g[other][:],
                    op=mybir.AluOpType.mult)
                nc.vector.tensor_reduce(
                    out=dsq[d][:, ci * GJ:(ci + 1) * GJ],
                    in_=g[other][:],
                    axis=mybir.AxisListType.X,
                    op=mybir.AluOpType.add)

        # epilogue: d^2 = ssum - 2*dot, clamp, sqrt, hinges, reductions
        dist = {}
        for d in ("ap", "an"):
            t = dsq[d]
            nc.vector.tensor_scalar_mul(t[:], t[:], -2.0)
            nc.vector.tensor_tensor(
                out=t[:], in0=t[:], in1=scal_sb[f"ssum_{d}"][:],
                op=mybir.AluOpType.add)
            nc.vector.tensor_scalar_max(t[:], t[:], 0.0)
            nc.scalar.activation(
                out=t[:], in_=t[:],
                func=mybir.ActivationFunctionType.Sqrt, bias=eps_sb[:])
            dist[d] = t

        pos = epi_pool.tile([128, ROWS], f32, tag="pos")
        nc.vector.tensor_tensor(
            out=pos[:], in0=dist["ap"][:], in1=scal_sb["bm"][:],
            op=mybir.AluOpType.subtract)
        nc.vector.tensor_scalar_max(pos[:], pos[:], 0.0)
        neg = epi_pool.tile([128, ROWS], f32, tag="neg")
        nc.vector.tensor_tensor(
            out=neg[:], in0=scal_sb["bp"][:], in1=dist["an"][:],
            op=mybir.AluOpType.subtract)
        nc.vector.tensor_scalar_max(neg[:], neg[:], 0.0)
        z = epi_pool.tile([128, ROWS], f32, tag="z")
        nc.vector.tensor_tensor(
            out=z[:], in0=pos[:], in1=neg[:], op=mybir.AluOpType.add)
        ind = epi_pool.tile([128, ROWS], f32, tag="ind")
        nc.vector.tensor_scalar(
            out=ind[:], in0=z[:], scalar1=0.0, scalar2=None,
            op0=mybir.AluOpType.is_gt)
        outsb = epi_pool.tile([128, 2], f32, tag="outsb")
        nc.vector.tensor_reduce(
            out=outsb[:, 0:1], in_=z[:], axis=mybir.AxisListType.X,
            op=mybir.AluOpType.add)
        nc.vector.tensor_reduce(
            out=outsb[:, 1:2], in_=ind[:], axis=mybir.AxisListType.X,
            op=mybir.AluOpType.add)
        nc.sync.dma_start(outp[:], outsb[:])

    nc.compile()
    return nc


def _prep_inputs(batch, beta, labels, triplets):
    batch = np.asarray(batch, dtype=np.float32)
    beta = np.asarray(beta, dtype=np.float32)
    labels = np.asarray(labels).astype(np.int64)
    triplets = np.asarray(triplets).astype(np.int64)

    bt_bf = batch.astype(ml_dtypes.bfloat16)
    s = (bt_bf.astype(np.float32) ** 2).sum(axis=1, dtype=np.float64)
    s = s.astype(np.float32)

    ia, ip, iN = triplets[:, 0], triplets[:, 1], triplets[:, 2]
    b = beta[labels[ia]].astype(np.float32)          # [T]
    ssum_ap = (s[ia] + s[ip]).astype(np.float32)
    ssum_an = (s[ia] + s[iN]).astype(np.float32)
    bm = (b - MARGIN).astype(np.float32)
    bp = (b + MARGIN).astype(np.float32)

    in_maps = []
    for core in range(N_CORES):
        sl = slice(core * T_LOC, (core + 1) * T_LOC)
        # triplet t=(p, f) at p*ROWS+f; idx columns: [a cols | p cols | n cols]
        idx_arr = np.concatenate(
            [col[sl].reshape(128, ROWS) for col in (ia, ip, iN)],
            axis=1).astype(np.int32)
        in_maps.append({
            "bt": bt_bf,
            "idx": np.ascontiguousarray(idx_arr),
            "ssum_ap": ssum_ap[sl].reshape(128, ROWS),
            "ssum_an": ssum_an[sl].reshape(128, ROWS),
            "bm": bm[sl].reshape(128, ROWS),
            "bp": bp[sl].reshape(128, ROWS),
        })
    return in_maps


def _finalize(results):
    total = np.float64(0.0)
    cnt = np.float64(0.0)
    for r in results:
        total += r["out"][:, 0].astype(np.float64).sum()
        cnt += r["out"][:, 1].astype(np.float64).sum()
    total = np.float32(total)
    cnt = np.float32(cnt)
    if cnt > 0.0:
        loss = total / max(cnt, np.float32(1.0))
    else:
        loss = total
    return np.float32(loss)


def run_hw(batch, beta, labels, triplets, trace=False, **kw):
    if "nc" not in _CACHE:
        _CACHE["nc"] = _build_nc()
    nc = _CACHE["nc"]
    in_maps = _prep_inputs(batch, beta, labels, triplets)
    res = run_bass_kernel_spmd(nc, in_maps, list(range(N_CORES)), trace=trace, **kw)
    return _finalize(res.results), res


def kernel(batch, beta, labels, triplets):
    loss, _ = run_hw(batch, beta, labels, triplets)
    return loss



# revision 3
# speedup vs baseline: 4.1097x; 4.1097x over previous
"""Margin-based triplet criterion (loss_fn) on 8 TRN2 NeuronCores.

Strategy (anchor-block sharding; each core owns B/8 = 512 anchor rows):
  - Host: cast batch to bf16, build bT [4, 128, 4096] (D-major transpose,
    K-chunked) replicated to all cores, and aT [128, 4, 512] = the core's
    anchor-block columns (matmul stationary). Re-sort triplets to the core
    owning their anchor (sum over triplets is permutation invariant), pad
    with masked dummies (bm=+LARGE, bp=-LARGE => zero contribution).
    Precompute per-triplet ssum = s[ia]+s[ip|n] (f32), hinge offsets
    bm/bp, and flat gather indices al*4096 + ip|n into the core's Gram
    block.
  - Device: PE computes the Gram block G = aT.T @ bT (512 anchors x 4096)
    as 128 bank-sized matmuls (K=512 in 4 chunks, N=512 per PSUM bank);
    ACT/DVE alternate downconverting PSUM f32 -> bf16 SBUF tiles which are
    DMA'd to a DRAM scratch tile. One indirect (SWDGE) gather pulls the
    2*NCOLS per-partition dot products g = G[al, ip|n] as scalars.
    Epilogue: d^2 = ssum - 2g (clamped), d = sqrt(d^2 + eps) on ACT,
    hinges pos = relu(d_ap - bm), neg = relu(bp - d_an), z = pos + neg,
    indicator z > 0, free-dim reductions -> [128, 2] (sum, count).
  - Host: sum the 8x128 partials, loss = total / max(count, 1) if count>0.
"""

import numpy as np
import ml_dtypes
from contextlib import ExitStack

import concourse.bass as bass
import concourse.bacc as bacc
import concourse.tile as tile
from concourse import mybir
from concourse.bass_utils import run_bass_kernel_spmd

N_CORES = 8
B, D, T, C = 4096, 512, 65536, 100
A_LOC = B // N_CORES            # 512 anchors per core
KCH = 4                         # K chunks of 128 (D = 512)
M_TILES = A_LOC // 128          # 4 anchor tiles per core
NBANK = 512                     # matmul N per PSUM bank (f32)
HALF = 2048                     # columns per copy/store unit (4 banks)
NCOLS = 68                      # padded triplet columns per partition
NC2 = 2 * NCOLS                 # ap|an concatenated
MARGIN = 0.2
EPS = 1e-8
LARGE = 4e6

f32 = mybir.dt.float32
bf16 = mybir.dt.bfloat16
i32 = mybir.dt.int32

_CACHE = {}


def _build_nc():
    nc = bacc.Bacc(
        "TRN2", target_bir_lowering=False, debug=False,
        enable_asserts=False, num_devices=N_CORES,
    )
    bT = nc.dram_tensor("bT", [KCH, 128, B], bf16, kind="ExternalInput")
    aT = nc.dram_tensor("aT", [128, KCH, A_LOC], bf16, kind="ExternalInput")
    gidx = nc.dram_tensor("gidx", [128, NC2], i32, kind="ExternalInput")
    ssum = nc.dram_tensor("ssum", [128, NC2], f32, kind="ExternalInput")
    bmp = nc.dram_tensor("bmp", [128, NC2], f32, kind="ExternalInput")
    outp = nc.dram_tensor("out", [128, 2], f32, kind="ExternalOutput")

    with tile.TileContext(nc) as tc, ExitStack() as ctx:
        const_pool = ctx.enter_context(tc.tile_pool(name="const", bufs=1))
        psum_pool = ctx.enter_context(tc.tile_pool(name="ps", bufs=2, space="PSUM"))
        gsb_pool = ctx.enter_context(tc.tile_pool(name="gsb", bufs=3))
        gdram_pool = ctx.enter_context(tc.tile_pool(name="gdram", bufs=1, space="DRAM"))
        epi_pool = ctx.enter_context(tc.tile_pool(name="epi", bufs=1))

        aT_sb = const_pool.tile([128, KCH, A_LOC], bf16)
        nc.sync.dma_start(aT_sb[:], aT[:])
        bT_sb = const_pool.tile([128, KCH, B], bf16)
        for k in range(KCH):
            nc.sync.dma_start(bT_sb[:, k, :], bT[k, :, :])

        eps_sb = const_pool.tile([128, 1], f32)
        nc.vector.memset(eps_sb[:], EPS)
        idx_sb = const_pool.tile([128, NC2], i32)
        nc.sync.dma_start(idx_sb[:], gidx[:])
        ssum_sb = const_pool.tile([128, NC2], f32)
        nc.sync.dma_start(ssum_sb[:], ssum[:])
        bmp_sb = const_pool.tile([128, NC2], f32)
        nc.sync.dma_start(bmp_sb[:], bmp[:])

        g_t = gdram_pool.tile([A_LOC, B], bf16)

        # Gram block: units of (anchor tile m, column half h)
        for m in range(M_TILES):
            for h in range(B // HALF):
                ps = psum_pool.tile([128, HALF // NBANK, NBANK], f32, tag="ps")
                for b in range(HALF // NBANK):
                    for k in range(KCH):
                        nc.tensor.matmul(
                            ps[:, b, :],
                            lhsT=aT_sb[:, k, m * 128:(m + 1) * 128],
                            rhs=bT_sb[:, k, h * HALF + b * NBANK:
                                      h * HALF + (b + 1) * NBANK],
                            start=(k == 0), stop=(k == KCH - 1),
                        )
                gsb = gsb_pool.tile([128, HALF], bf16, tag="gsb")
                if (m * (B // HALF) + h) % 2 == 0:
                    nc.scalar.copy(out=gsb[:], in_=ps[:, :, :])
                else:
                    nc.vector.tensor_copy(out=gsb[:], in_=ps[:, :, :])
                nc.sync.dma_start(
                    g_t[m * 128:(m + 1) * 128, h * HALF:(h + 1) * HALF], gsb[:])

        # Per-triplet dot products: scalar gather from the DRAM Gram block
        g_vals = epi_pool.tile([128, NC2], bf16, tag="gv")
        nc.gpsimd.indirect_dma_start(
            out=g_vals[:],
            out_offset=None,
            in_=g_t[:],
            in_offset=bass.IndirectOffsetOnAxis(ap=idx_sb[:], axis=1),
        )

        # d^2 = ssum - 2*g, clamped; d = sqrt(d^2 + eps)
        dsq = epi_pool.tile([128, NC2], f32, tag="dsq")
        nc.vector.scalar_tensor_tensor(
            out=dsq[:], in0=g_vals[:], scalar=-2.0, in1=ssum_sb[:],
            op0=mybir.AluOpType.mult, op1=mybir.AluOpType.add)
        nc.vector.tensor_scalar_max(dsq[:], dsq[:], 0.0)
        dist = epi_pool.tile([128, NC2], f32, tag="dist")
        nc.scalar.activation(
            out=dist[:], in_=dsq[:],
            func=mybir.ActivationFunctionType.Sqrt, bias=eps_sb[:])

        # hinges: pos = relu(d_ap - bm), neg = relu(bp - d_an)
        hing = epi_pool.tile([128, NC2], f32, tag="hing")
        nc.vector.tensor_tensor(
            out=hing[:, 0:NCOLS], in0=dist[:, 0:NCOLS], in1=bmp_sb[:, 0:NCOLS],
            op=mybir.AluOpType.subtract)
        nc.vector.tensor_tensor(
            out=hing[:, NCOLS:NC2], in0=bmp_sb[:, NCOLS:NC2],
            in1=dist[:, NCOLS:NC2], op=mybir.AluOpType.subtract)
        nc.vector.tensor_scalar_max(hing[:], hing[:], 0.0)
        z = epi_pool.tile([128, NCOLS], f32, tag="z")
        nc.vector.tensor_tensor(
            out=z[:], in0=hing[:, 0:NCOLS], in1=hing[:, NCOLS:NC2],
            op=mybir.AluOpType.add)
        ind = epi_pool.tile([128, NCOLS], f32, tag="ind")
        nc.vector.tensor_scalar(
            out=ind[:], in0=z[:], scalar1=0.0, scalar2=None,
            op0=mybir.AluOpType.is_gt)
        outsb = epi_pool.tile([128, 2], f32, tag="outsb")
        nc.vector.tensor_reduce(
            out=outsb[:, 0:1], in_=z[:], axis=mybir.AxisListType.X,
            op=mybir.AluOpType.add)
        nc.vector.tensor_reduce(
            out=outsb[:, 1:2], in_=ind[:], axis=mybir.AxisListType.X,
            op=mybir.AluOpType.add)
        nc.sync.dma_start(outp[:], outsb[:])

    nc.compile()
    return nc


def _prep_inputs(batch, beta, labels, triplets):
    batch = np.asarray(batch, dtype=np.float32)
    beta = np.asarray(beta, dtype=np.float32)
    labels = np.asarray(labels).astype(np.int64)
    triplets = np.asarray(triplets).astype(np.int64)

    bq = batch.astype(ml_dtypes.bfloat16)
    bqf = bq.astype(np.float32)
    s = (bqf.astype(np.float64) ** 2).sum(axis=1).astype(np.float32)

    # bT[k, d, j] = bq[j, 128k + d]
    bT_all = np.ascontiguousarray(bq.T.reshape(KCH, 128, B))

    ia, ip, iN = triplets[:, 0], triplets[:, 1], triplets[:, 2]
    core = ia // A_LOC
    al = ia % A_LOC
    b = beta[labels[ia]].astype(np.float32)

    CAP = 128 * NCOLS
    in_maps = []
    for c in range(N_CORES):
        sel = np.nonzero(core == c)[0]
        n = len(sel)
        assert n <= CAP, f"core {c} overflow: {n} > {CAP}"

        def packi(vals, fill):
            arr = np.full(CAP, fill, dtype=np.int32)
            arr[:n] = vals
            return arr.reshape(NCOLS, 128).T  # (p, f) = arr[f*128+p]

        def packf(vals, fill):
            arr = np.full(CAP, fill, dtype=np.float32)
            arr[:n] = vals
            return arr.reshape(NCOLS, 128).T

        alc = al[sel]
        gidx = np.concatenate(
            [packi(alc * B + ip[sel], 0), packi(alc * B + iN[sel], 0)], axis=1)
        ssum = np.concatenate(
            [packf(s[ia[sel]] + s[ip[sel]], 0.0),
             packf(s[ia[sel]] + s[iN[sel]], 0.0)], axis=1)
        # first half bm (pad +LARGE kills pos), second half bp (pad -LARGE
        # kills neg: device computes relu(bp - d))
        bmp = np.concatenate(
            [packf(b[sel] - MARGIN, LARGE), packf(b[sel] + MARGIN, -LARGE)],
            axis=1)

        aTc = np.ascontiguousarray(
            bT_all[:, :, c * A_LOC:(c + 1) * A_LOC].transpose(1, 0, 2))

        in_maps.append({
            "bT": bT_all,
            "aT": aTc,
            "gidx": np.ascontiguousarray(gidx),
            "ssum": np.ascontiguousarray(ssum),
            "bmp": np.ascontiguousarray(bmp),
        })
    return in_maps


def _finalize(results):
    total = np.float64(0.0)
    cnt = np.float64(0.0)
    for r in results:
        total += r["out"][:, 0].astype(np.float64).sum()
        cnt += r["out"][:, 1].astype(np.float64).sum()
    total = np.float32(total)
    cnt = np.float32(cnt)
    if cnt > 0.0:
        loss = total / max(cnt, np.float32(1.0))
    else:
        loss = total
    return np.float32(loss)


def run_hw(batch, beta, labels, triplets, trace=False, **kw):
    if "nc" not in _CACHE:
        _CACHE["nc"] = _build_nc()
    nc = _CACHE["nc"]
    in_maps = _prep_inputs(batch, beta, labels, triplets)
    res = run_bass_kernel_spmd(nc, in_maps, list(range(N_CORES)), trace=trace, **kw)
    return _finalize(res.results), res


def kernel(batch, beta, labels, triplets):
    loss, _ = run_hw(batch, beta, labels, triplets)
    return loss


# revision 6
# speedup vs baseline: 6.1786x; 1.5034x over previous
"""Margin-based triplet criterion (loss_fn) on 8 TRN2 NeuronCores.

Strategy (anchor-block sharding; each core owns B/8 = 512 anchor rows):
  - Host: quantize batch*0.5 to fp8 e4m3 (so the PE Gram values G/4 stay in
    e4m3 range for the fp8 store), build bT [4, 128, 4096] (D-major
    transpose, K-chunked, replicated) and aT [128, 4, 512] = the core's
    anchor-block columns (stationary). Re-sort triplets to the core owning
    their anchor (loss is a permutation-invariant sum), pad with masked
    dummies (bm=+LARGE, bp=-LARGE => zero contribution). Precompute
    per-triplet ssum = s[ia]+s[ip|n] (f32, from the dequantized rows),
    hinge offsets bm/bp, and flat gather indices into the Gram block
    laid out [p, m, col] (anchor a -> partition a%128, sub-row a//128).
  - Device: PE computes the Gram block (512 anchors x 4096) with fp8
    DoubleRow matmuls (K=256 per pass, N=512 per PSUM bank; 64 matmuls).
    DVE/ACT/Pool alternate downconverting PSUM f32 -> fp8 SBUF staging
    tiles [128, 4096] which are DMA'd (4 stores) to a DRAM scratch tile.
    One indirect (SWDGE) gather pulls the 2*NCOLS per-partition dot
    products as scalars. Epilogue: d^2 = ssum - 8*g (clamped), d =
    sqrt(d^2 + eps) on ACT, hinges pos = relu(d_ap - bm), neg =
    relu(bp - d_an), z = pos + neg, indicator z > 0, free-dim reductions
    -> [128, 2] (sum, count) per core.
  - Host: sum the 8x128 partials, loss = total / max(count, 1) if count>0.
"""

import numpy as np
import ml_dtypes
from contextlib import ExitStack

import concourse.bass as bass
import concourse.bacc as bacc
import concourse.tile as tile
from concourse import mybir
from concourse.bass_utils import run_bass_kernel_spmd

N_CORES = 8
B, D, T, C = 4096, 512, 65536, 100
A_LOC = B // N_CORES            # 512 anchors per core
KCH = 4                         # K chunks of 128 (D = 512)
M_TILES = A_LOC // 128          # 4 anchor tiles per core
NBANK = 512                     # matmul N per PSUM bank (f32)
HALF = 2048                     # columns per copy unit (4 banks)
NCOLS = 68                      # padded triplet columns per partition
NC2 = 2 * NCOLS                 # ap|an concatenated
MARGIN = 0.2
EPS = 1e-8
LARGE = 4e6

f32 = mybir.dt.float32
bf16 = mybir.dt.bfloat16
fp8 = mybir.dt.float8e4
i32 = mybir.dt.int32
FP8NP = ml_dtypes.float8_e4m3

_CACHE = {}


def _build_nc():
    nc = bacc.Bacc(
        "TRN2", target_bir_lowering=False, debug=False,
        enable_asserts=False, num_devices=N_CORES,
    )
    bT = nc.dram_tensor("bT", [KCH, 128, B], fp8, kind="ExternalInput")
    aT = nc.dram_tensor("aT", [128, KCH, A_LOC], fp8, kind="ExternalInput")
    gidx = nc.dram_tensor("gidx", [128, NC2], i32, kind="ExternalInput")
    ssum = nc.dram_tensor("ssum", [128, NC2], f32, kind="ExternalInput")
    bmp = nc.dram_tensor("bmp", [128, NC2], f32, kind="ExternalInput")
    outp = nc.dram_tensor("out", [128, 2], f32, kind="ExternalOutput")

    with tile.TileContext(nc) as tc, ExitStack() as ctx:
        const_pool = ctx.enter_context(tc.tile_pool(name="const", bufs=1))
        psum_pool = ctx.enter_context(tc.tile_pool(name="ps", bufs=2, space="PSUM"))
        gsb_pool = ctx.enter_context(tc.tile_pool(name="gsb", bufs=2))
        gdram_pool = ctx.enter_context(tc.tile_pool(name="gdram", bufs=1, space="DRAM"))
        epi_pool = ctx.enter_context(tc.tile_pool(name="epi", bufs=1))

        aT_sb = const_pool.tile([128, KCH, A_LOC], fp8)
        nc.sync.dma_start(aT_sb[:], aT[:])
        bT_sb = const_pool.tile([128, KCH, B], fp8)
        for k in range(KCH):
            nc.sync.dma_start(bT_sb[:, k, :], bT[k, :, :])

        eps_sb = const_pool.tile([128, 1], f32)
        nc.vector.memset(eps_sb[:], EPS)
        idx_sb = const_pool.tile([128, NC2], i32)
        nc.sync.dma_start(idx_sb[:], gidx[:])
        ssum_sb = const_pool.tile([128, NC2], f32)
        nc.sync.dma_start(ssum_sb[:], ssum[:])
        bmp_sb = const_pool.tile([128, NC2], f32)
        nc.sync.dma_start(bmp_sb[:], bmp[:])

        # Gram scratch, laid out [p, m, col]: anchor a = 128m + p
        g_t = gdram_pool.tile([128, M_TILES * B], fp8)

        # Gram block: per anchor tile m, 2 column-half units of 4 banks
        copy_engines = [nc.scalar, nc.vector]
        for m in range(M_TILES):
            gsb = gsb_pool.tile([128, B], fp8, tag="gsb")
            for h in range(B // HALF):
                ps = psum_pool.tile([128, HALF // NBANK, NBANK], f32, tag="ps")
                # k2-outer so PE only waits on bT chunks 2*k2, 2*k2+1
                for k2 in range(KCH // 2):
                    for b in range(HALF // NBANK):
                        nc.tensor.matmul(
                            ps[:, b, :],
                            lhsT=aT_sb[:, 2 * k2:2 * k2 + 2,
                                       m * 128:(m + 1) * 128],
                            rhs=bT_sb[:, 2 * k2:2 * k2 + 2,
                                      h * HALF + b * NBANK:
                                      h * HALF + (b + 1) * NBANK],
                            start=(k2 == 0), stop=(k2 == KCH // 2 - 1),
                            perf_mode=mybir.MatmulPerfMode.DoubleRow,
                        )
                eng = copy_engines[(m * (B // HALF) + h) % 2]
                if eng is nc.scalar:
                    eng.copy(out=gsb[:, h * HALF:(h + 1) * HALF], in_=ps[:, :, :])
                else:
                    eng.tensor_copy(out=gsb[:, h * HALF:(h + 1) * HALF],
                                    in_=ps[:, :, :])
            nc.sync.dma_start(g_t[:, m * B:(m + 1) * B], gsb[:])

        # Per-triplet dot products: scalar gather from the DRAM Gram block
        g_vals = epi_pool.tile([128, NC2], fp8, tag="gv")
        nc.gpsimd.indirect_dma_start(
            out=g_vals[:],
            out_offset=None,
            in_=g_t[:],
            in_offset=bass.IndirectOffsetOnAxis(ap=idx_sb[:], axis=1),
        )

        # d^2 = ssum - 8*g (batch was scaled by 0.5 => psum held G/4)
        dsq = epi_pool.tile([128, NC2], f32, tag="dsq")
        nc.vector.scalar_tensor_tensor(
            out=dsq[:], in0=g_vals[:], scalar=-8.0, in1=ssum_sb[:],
            op0=mybir.AluOpType.mult, op1=mybir.AluOpType.add)
        nc.vector.tensor_scalar_max(dsq[:], dsq[:], 0.0)
        dist = epi_pool.tile([128, NC2], f32, tag="dist")
        nc.scalar.activation(
            out=dist[:], in_=dsq[:],
            func=mybir.ActivationFunctionType.Sqrt, bias=eps_sb[:])

        # hinges: pos = relu(d_ap - bm), neg = relu(bp - d_an)
        hing = epi_pool.tile([128, NC2], f32, tag="hing")
        nc.vector.tensor_tensor(
            out=hing[:, 0:NCOLS], in0=dist[:, 0:NCOLS], in1=bmp_sb[:, 0:NCOLS],
            op=mybir.AluOpType.subtract)
        nc.vector.tensor_tensor(
            out=hing[:, NCOLS:NC2], in0=bmp_sb[:, NCOLS:NC2],
            in1=dist[:, NCOLS:NC2], op=mybir.AluOpType.subtract)
        nc.vector.tensor_scalar_max(hing[:], hing[:], 0.0)
        z = epi_pool.tile([128, NCOLS], f32, tag="z")
        nc.vector.tensor_tensor(
            out=z[:], in0=hing[:, 0:NCOLS], in1=hing[:, NCOLS:NC2],
            op=mybir.AluOpType.add)
        ind = epi_pool.tile([128, NCOLS], f32, tag="ind")
        nc.vector.tensor_scalar(
            out=ind[:], in0=z[:], scalar1=0.0, scalar2=None,
            op0=mybir.AluOpType.is_gt)
        outsb = epi_pool.tile([128, 2], f32, tag="outsb")
        nc.vector.tensor_reduce(
            out=outsb[:, 0:1], in_=z[:], axis=mybir.AxisListType.X,
            op=mybir.AluOpType.add)
        nc.vector.tensor_reduce(
            out=outsb[:, 1:2], in_=ind[:], axis=mybir.AxisListType.X,
            op=mybir.AluOpType.add)
        nc.sync.dma_start(outp[:], outsb[:])

    nc.compile()
    return nc


def _prep_inputs(batch, beta, labels, triplets):
    batch = np.asarray(batch, dtype=np.float32)
    beta = np.asarray(beta, dtype=np.float32)
    labels = np.asarray(labels).astype(np.int64)
    triplets = np.asarray(triplets).astype(np.int64)

    q = (0.5 * batch).astype(FP8NP)          # device rows (scaled by 1/2)
    qf = q.astype(np.float32)
    # effective embedding is 2*q; s = |2q|^2
    s = 4.0 * (qf.astype(np.float64) ** 2).sum(axis=1)
    s = s.astype(np.float32)

    # bT[k, d, j] = q[j, 128k + d]
    bT_all = np.ascontiguousarray(q.T.reshape(KCH, 128, B))

    ia, ip, iN = triplets[:, 0], triplets[:, 1], triplets[:, 2]
    core = ia // A_LOC
    al = ia % A_LOC
    b = beta[labels[ia]].astype(np.float32)

    # Gram scratch layout [p, m, col]: flat = (al%128)*(M_TILES*B) + (al//128)*B + col
    CAP = 128 * NCOLS
    in_maps = []
    for c in range(N_CORES):
        sel = np.nonzero(core == c)[0]
        n = len(sel)
        assert n <= CAP, f"core {c} overflow: {n} > {CAP}"

        def packi(vals, fill):
            arr = np.full(CAP, fill, dtype=np.int64)
            arr[:n] = vals
            return arr.reshape(NCOLS, 128).T.astype(np.int32)

        def packf(vals, fill):
            arr = np.full(CAP, fill, dtype=np.float32)
            arr[:n] = vals
            return arr.reshape(NCOLS, 128).T

        alc = al[sel]
        base = (alc % 128) * (M_TILES * B) + (alc // 128) * B
        gidx = np.concatenate(
            [packi(base + ip[sel], 0), packi(base + iN[sel], 0)], axis=1)
        ssum = np.concatenate(
            [packf(s[ia[sel]] + s[ip[sel]], 0.0),
             packf(s[ia[sel]] + s[iN[sel]], 0.0)], axis=1)
        # first half bm (pad +LARGE kills pos), second half bp (pad -LARGE
        # kills neg: device computes relu(bp - d))
        bmp = np.concatenate(
            [packf(b[sel] - MARGIN, LARGE), packf(b[sel] + MARGIN, -LARGE)],
            axis=1)

        aTc = np.ascontiguousarray(
            bT_all[:, :, c * A_LOC:(c + 1) * A_LOC].transpose(1, 0, 2))

        in_maps.append({
            "bT": bT_all,
            "aT": aTc,
            "gidx": np.ascontiguousarray(gidx),
            "ssum": np.ascontiguousarray(ssum),
            "bmp": np.ascontiguousarray(bmp),
        })
    return in_maps


def _finalize(results):
    total = np.float64(0.0)
    cnt = np.float64(0.0)
    for r in results:
        total += r["out"][:, 0].astype(np.float64).sum()
        cnt += r["out"][:, 1].astype(np.float64).sum()
    total = np.float32(total)
    cnt = np.float32(cnt)
    if cnt > 0.0:
        loss = total / max(cnt, np.float32(1.0))
    else:
        loss = total
    return np.float32(loss)


def run_hw(batch, beta, labels, triplets, trace=False, **kw):
    if "nc" not in _CACHE:
        _CACHE["nc"] = _build_nc()
    nc = _CACHE["nc"]
    in_maps = _prep_inputs(batch, beta, labels, triplets)
    res = run_bass_kernel_spmd(nc, in_maps, list(range(N_CORES)), trace=trace, **kw)
    return _finalize(res.results), res


def kernel(batch, beta, labels, triplets):
    loss, _ = run_hw(batch, beta, labels, triplets)
    return loss


# revision 13
# speedup vs baseline: 6.3034x; 1.0202x over previous
"""Margin-based triplet criterion (loss_fn) on 8 TRN2 NeuronCores.

Strategy (anchor-block sharding; each core owns B/8 = 512 anchor rows):
  - Host: quantize batch*0.5 to fp8 e4m3 (so the PE Gram values G/4 stay in
    e4m3 range for the fp8 store), build bT [4, 128, 4096] (D-major
    transpose, K-chunked, replicated) and aT [128, 4, 512] = the core's
    anchor-block columns (stationary). Re-sort triplets to the core owning
    their anchor (loss is a permutation-invariant sum), pad with masked
    dummies (bm=+LARGE, bp=-LARGE => zero contribution). Precompute
    per-triplet ssum = s[ia]+s[ip|n] (f32, from the dequantized rows),
    hinge offsets bm/bp, and flat gather indices into the Gram block
    laid out [p, m, col] (anchor a -> partition a%128, sub-row a//128).
  - Device: PE computes the Gram block (512 anchors x 4096) with fp8
    DoubleRow matmuls (K=256 per pass, N=512 per PSUM bank; 64 matmuls).
    DVE/ACT/Pool alternate downconverting PSUM f32 -> fp8 SBUF staging
    tiles [128, 4096] which are DMA'd (4 stores) to a DRAM scratch tile.
    One indirect (SWDGE) gather pulls the 2*NCOLS per-partition dot
    products as scalars. Epilogue: d^2 = ssum - 8*g (clamped), d =
    sqrt(d^2 + eps) on ACT, hinges pos = relu(d_ap - bm), neg =
    relu(bp - d_an), z = pos + neg, indicator z > 0, free-dim reductions
    -> [128, 2] (sum, count) per core.
  - Host: sum the 8x128 partials, loss = total / max(count, 1) if count>0.
"""

import numpy as np
import ml_dtypes
from contextlib import ExitStack

import concourse.bass as bass
import concourse.bacc as bacc
import concourse.tile as tile
from concourse import mybir
from concourse.bass_utils import run_bass_kernel_spmd

N_CORES = 8
B, D, T, C = 4096, 512, 65536, 100
A_LOC = B // N_CORES            # 512 anchors per core
KCH = 4                         # K chunks of 128 (D = 512)
M_TILES = A_LOC // 128          # 4 anchor tiles per core
NBANK = 512                     # matmul N per PSUM bank (f32)
HALF = 2048                     # columns per copy unit (4 banks)
NCOLS = 68                      # padded triplet columns per partition
NC2 = 2 * NCOLS                 # ap|an concatenated
MARGIN = 0.2
EPS = 1e-8
LARGE = 4e6

f32 = mybir.dt.float32
bf16 = mybir.dt.bfloat16
fp8 = mybir.dt.float8e4
i32 = mybir.dt.int32
FP8NP = ml_dtypes.float8_e4m3

_CACHE = {}


def _build_nc():
    nc = bacc.Bacc(
        "TRN2", target_bir_lowering=False, debug=False,
        enable_asserts=False, num_devices=N_CORES,
    )
    bT = nc.dram_tensor("bT", [KCH, 128, B], fp8, kind="ExternalInput")
    aT = nc.dram_tensor("aT", [128, KCH, A_LOC], fp8, kind="ExternalInput")
    gidx = nc.dram_tensor("gidx", [128, NC2], i32, kind="ExternalInput")
    ssum = nc.dram_tensor("ssum", [128, NC2], f32, kind="ExternalInput")
    bmp = nc.dram_tensor("bmp", [128, NC2], f32, kind="ExternalInput")
    outp = nc.dram_tensor("out", [128, 2], f32, kind="ExternalOutput")

    with tile.TileContext(nc) as tc, ExitStack() as ctx:
        const_pool = ctx.enter_context(tc.tile_pool(name="const", bufs=1))
        psum_pool = ctx.enter_context(tc.tile_pool(name="ps", bufs=2, space="PSUM"))
        gsb_pool = ctx.enter_context(tc.tile_pool(name="gsb", bufs=4))
        gdram_pool = ctx.enter_context(tc.tile_pool(name="gdram", bufs=1, space="DRAM"))
        epi_pool = ctx.enter_context(tc.tile_pool(name="epi", bufs=1))

        aT_sb = const_pool.tile([128, KCH, A_LOC], fp8)
        nc.sync.dma_start(aT_sb[:], aT[:])
        bT_sb = const_pool.tile([128, KCH, B], fp8)
        for k in range(KCH):
            nc.sync.dma_start(bT_sb[:, k, :], bT[k, :, :])

        eps_sb = const_pool.tile([128, 1], f32)
        nc.vector.memset(eps_sb[:], EPS)
        idx_sb = const_pool.tile([128, NC2], i32)
        nc.sync.dma_start(idx_sb[:], gidx[:])
        ssum_sb = const_pool.tile([128, NC2], f32)
        nc.sync.dma_start(ssum_sb[:], ssum[:])
        bmp_sb = const_pool.tile([128, NC2], f32)
        nc.sync.dma_start(bmp_sb[:], bmp[:])

        # Gram scratch, laid out [p, m, col]: anchor a = 128m + p
        g_t = gdram_pool.tile([128, M_TILES * B], fp8)

        # Gram block: per anchor tile m, 2 column-half units of 4 banks.
        # Both copy engines split each unit (halves PSUM release latency).
        for m in range(M_TILES):
            for h in range(B // HALF):
                ps = psum_pool.tile([128, HALF // NBANK, NBANK], f32, tag="ps")
                # k2-outer so PE only waits on bT chunks 2*k2, 2*k2+1
                for k2 in range(KCH // 2):
                    for b in range(HALF // NBANK):
                        nc.tensor.matmul(
                            ps[:, b, :],
                            lhsT=aT_sb[:, 2 * k2:2 * k2 + 2,
                                       m * 128:(m + 1) * 128],
                            rhs=bT_sb[:, 2 * k2:2 * k2 + 2,
                                      h * HALF + b * NBANK:
                                      h * HALF + (b + 1) * NBANK],
                            start=(k2 == 0), stop=(k2 == KCH // 2 - 1),
                            perf_mode=mybir.MatmulPerfMode.DoubleRow,
                        )
                gsb = gsb_pool.tile([128, HALF], fp8, tag="gsb")
                nc.scalar.copy(out=gsb[:, 0:HALF // 2], in_=ps[:, 0:2, :])
                nc.vector.tensor_copy(out=gsb[:, HALF // 2:HALF],
                                      in_=ps[:, 2:4, :])
                nc.sync.dma_start(
                    g_t[:, m * B + h * HALF:m * B + (h + 1) * HALF], gsb[:])

        # Per-triplet dot products: scalar gather from the DRAM Gram block
        g_vals = epi_pool.tile([128, NC2], fp8, tag="gv")
        nc.gpsimd.indirect_dma_start(
            out=g_vals[:],
            out_offset=None,
            in_=g_t[:],
            in_offset=bass.IndirectOffsetOnAxis(ap=idx_sb[:], axis=1),
        )

        # d^2 = ssum - 8*g = -8*(g - ssum/8); host passes ssum/8.
        dif = epi_pool.tile([128, NC2], f32, tag="dif")
        nc.vector.tensor_tensor(
            out=dif[:], in0=g_vals[:], in1=ssum_sb[:],
            op=mybir.AluOpType.subtract)
        dsq = epi_pool.tile([128, NC2], f32, tag="dsq")
        nc.vector.tensor_scalar(
            out=dsq[:], in0=dif[:], scalar1=-8.0, scalar2=0.0,
            op0=mybir.AluOpType.mult, op1=mybir.AluOpType.max)
        dist = epi_pool.tile([128, NC2], f32, tag="dist")
        nc.scalar.activation(
            out=dist[:], in_=dsq[:],
            func=mybir.ActivationFunctionType.Sqrt, bias=eps_sb[:])

        # hinges: pos = relu(d_ap - bm) (DVE), neg = relu(bp - d_an) (Pool)
        hing = epi_pool.tile([128, NC2], f32, tag="hing")
        nc.vector.tensor_tensor(
            out=hing[:, 0:NCOLS], in0=dist[:, 0:NCOLS], in1=bmp_sb[:, 0:NCOLS],
            op=mybir.AluOpType.subtract)
        nc.gpsimd.tensor_tensor(
            out=hing[:, NCOLS:NC2], in0=bmp_sb[:, NCOLS:NC2],
            in1=dist[:, NCOLS:NC2], op=mybir.AluOpType.subtract)
        nc.vector.tensor_scalar_max(hing[:], hing[:], 0.0)
        z = epi_pool.tile([128, NCOLS], f32, tag="z")
        outsb = epi_pool.tile([128, 2], f32, tag="outsb")
        nc.vector.tensor_tensor(
            out=z[:], in0=hing[:, 0:NCOLS], in1=hing[:, NCOLS:NC2],
            op=mybir.AluOpType.add)
        ind = epi_pool.tile([128, NCOLS], f32, tag="ind")
        nc.vector.tensor_scalar(
            out=ind[:], in0=z[:], scalar1=0.0, scalar2=None,
            op0=mybir.AluOpType.is_gt)
        nc.vector.tensor_reduce(
            out=outsb[:, 0:1], in_=z[:], axis=mybir.AxisListType.X,
            op=mybir.AluOpType.add)
        nc.vector.tensor_reduce(
            out=outsb[:, 1:2], in_=ind[:], axis=mybir.AxisListType.X,
            op=mybir.AluOpType.add)
        nc.sync.dma_start(outp[:], outsb[:])

    nc.compile()
    return nc


def _prep_inputs(batch, beta, labels, triplets):
    batch = np.asarray(batch, dtype=np.float32)
    beta = np.asarray(beta, dtype=np.float32)
    labels = np.asarray(labels).astype(np.int64)
    triplets = np.asarray(triplets).astype(np.int64)

    q = (0.5 * batch).astype(FP8NP)          # device rows (scaled by 1/2)
    qf = q.astype(np.float32)
    # effective embedding is 2*q; s = |2q|^2
    s = 4.0 * (qf.astype(np.float64) ** 2).sum(axis=1)
    s = s.astype(np.float32)

    # bT[k, d, j] = q[j, 128k + d]
    bT_all = np.ascontiguousarray(q.T.reshape(KCH, 128, B))

    ia, ip, iN = triplets[:, 0], triplets[:, 1], triplets[:, 2]
    core = ia // A_LOC
    al = ia % A_LOC
    b = beta[labels[ia]].astype(np.float32)

    # Gram scratch layout [p, m, col]: flat = (al%128)*(M_TILES*B) + (al//128)*B + col
    CAP = 128 * NCOLS
    in_maps = []
    for c in range(N_CORES):
        sel = np.nonzero(core == c)[0]
        n = len(sel)
        assert n <= CAP, f"core {c} overflow: {n} > {CAP}"

        def packi(vals, fill):
            arr = np.full(CAP, fill, dtype=np.int64)
            arr[:n] = vals
            return arr.reshape(NCOLS, 128).T.astype(np.int32)

        def packf(vals, fill):
            arr = np.full(CAP, fill, dtype=np.float32)
            arr[:n] = vals
            return arr.reshape(NCOLS, 128).T

        alc = al[sel]
        base = (alc % 128) * (M_TILES * B) + (alc // 128) * B
        gidx = np.concatenate(
            [packi(base + ip[sel], 0), packi(base + iN[sel], 0)], axis=1)
        # device computes d^2 = -8*(g - ssum/8): pass ssum/8
        ssum = np.concatenate(
            [packf((s[ia[sel]] + s[ip[sel]]) / 8.0, 0.0),
             packf((s[ia[sel]] + s[iN[sel]]) / 8.0, 0.0)], axis=1)
        # first half bm (pad +LARGE kills pos), second half bp (pad -LARGE
        # kills neg: device computes relu(bp - d))
        bmp = np.concatenate(
            [packf(b[sel] - MARGIN, LARGE), packf(b[sel] + MARGIN, -LARGE)],
            axis=1)

        aTc = np.ascontiguousarray(
            bT_all[:, :, c * A_LOC:(c + 1) * A_LOC].transpose(1, 0, 2))

        in_maps.append({
            "bT": bT_all,
            "aT": aTc,
            "gidx": np.ascontiguousarray(gidx),
            "ssum": np.ascontiguousarray(ssum),
            "bmp": np.ascontiguousarray(bmp),
        })
    return in_maps


def _finalize(results):
    total = np.float64(0.0)
    cnt = np.float64(0.0)
    for r in results:
        total += r["out"][:, 0].astype(np.float64).sum()
        cnt += r["out"][:, 1].astype(np.float64).sum()
    total = np.float32(total)
    cnt = np.float32(cnt)
    if cnt > 0.0:
        loss = total / max(cnt, np.float32(1.0))
    else:
        loss = total
    return np.float32(loss)


def run_hw(batch, beta, labels, triplets, trace=False, **kw):
    if "nc" not in _CACHE:
        _CACHE["nc"] = _build_nc()
    nc = _CACHE["nc"]
    in_maps = _prep_inputs(batch, beta, labels, triplets)
    res = run_bass_kernel_spmd(nc, in_maps, list(range(N_CORES)), trace=trace, **kw)
    return _finalize(res.results), res


def kernel(batch, beta, labels, triplets):
    loss, _ = run_hw(batch, beta, labels, triplets)
    return loss


# revision 17
# speedup vs baseline: 6.6244x; 1.0509x over previous
"""Margin-based triplet criterion (loss_fn) on 8 TRN2 NeuronCores.

Strategy (anchor-block sharding; each core owns B/8 = 512 anchor rows):
  - Host: quantize batch*0.5 to fp8 e4m3 (so the PE Gram values G/4 stay in
    e4m3 range for the fp8 store), build bT [4, 128, 4096] (D-major
    transpose, K-chunked, replicated) and aT [128, 4, 512] = the core's
    anchor-block columns (stationary). Re-sort triplets to the core owning
    their anchor (loss is a permutation-invariant sum), pad with masked
    dummies (bm=+LARGE, bp=-LARGE => zero contribution). Precompute
    per-triplet ssum = s[ia]+s[ip|n] (f32, from the dequantized rows),
    hinge offsets bm/bp, and flat gather indices into the Gram block
    laid out [p, m, col] (anchor a -> partition a%128, sub-row a//128).
  - Device: PE computes the Gram block (512 anchors x 4096) with fp8
    DoubleRow matmuls (K=256 per pass, N=512 per PSUM bank; 64 matmuls).
    DVE/ACT/Pool alternate downconverting PSUM f32 -> fp8 SBUF staging
    tiles [128, 4096] which are DMA'd (4 stores) to a DRAM scratch tile.
    One indirect (SWDGE) gather pulls the 2*NCOLS per-partition dot
    products as scalars. Epilogue: d^2 = ssum - 8*g (clamped), d =
    sqrt(d^2 + eps) on ACT, hinges pos = relu(d_ap - bm), neg =
    relu(bp - d_an), z = pos + neg, indicator z > 0, free-dim reductions
    -> [128, 2] (sum, count) per core.
  - Host: sum the 8x128 partials, loss = total / max(count, 1) if count>0.
"""

import numpy as np
import ml_dtypes
from contextlib import ExitStack

import concourse.bass as bass
import concourse.bacc as bacc
import concourse.tile as tile
from concourse import mybir
from concourse.bass_utils import run_bass_kernel_spmd

N_CORES = 8
B, D, T, C = 4096, 512, 65536, 100
A_LOC = B // N_CORES            # 512 anchors per core
KCH = 4                         # K chunks of 128 (D = 512)
M_TILES = A_LOC // 128          # 4 anchor tiles per core
NBANK = 512                     # matmul N per PSUM bank (f32)
HALF = 2048                     # columns per copy unit (4 banks)
NCOLS = 68                      # padded triplet columns per partition
NC2 = 2 * NCOLS                 # ap|an concatenated
MARGIN = 0.2
EPS = 1e-8
LARGE = 4e6

f32 = mybir.dt.float32
bf16 = mybir.dt.bfloat16
fp8 = mybir.dt.float8e4
i32 = mybir.dt.int32
FP8NP = ml_dtypes.float8_e4m3

_CACHE = {}


def _build_nc():
    nc = bacc.Bacc(
        "TRN2", target_bir_lowering=False, debug=False,
        enable_asserts=False, num_devices=N_CORES,
    )
    bT = nc.dram_tensor("bT", [KCH, 128, B], fp8, kind="ExternalInput")
    aT = nc.dram_tensor("aT", [128, KCH, A_LOC], fp8, kind="ExternalInput")
    gidx = nc.dram_tensor("gidx", [128, NC2], i32, kind="ExternalInput")
    ssum = nc.dram_tensor("ssum", [128, NC2], f32, kind="ExternalInput")
    bmp = nc.dram_tensor("bmp", [128, NC2], f32, kind="ExternalInput")
    outp = nc.dram_tensor("out", [128, 2], f32, kind="ExternalOutput")

    with tile.TileContext(nc) as tc, ExitStack() as ctx:
        const_pool = ctx.enter_context(tc.tile_pool(name="const", bufs=1))
        psum_pool = ctx.enter_context(tc.tile_pool(name="ps", bufs=2, space="PSUM"))
        gsb_pool = ctx.enter_context(tc.tile_pool(name="gsb", bufs=4))
        gdram_pool = ctx.enter_context(tc.tile_pool(name="gdram", bufs=1, space="DRAM"))
        epi_pool = ctx.enter_context(tc.tile_pool(name="epi", bufs=1))

        # Loads ordered so PE can start ASAP: stationary aT, then all 4 k
        # chunks of the first column half (enough for the h=0 units), then
        # the second half, then gather/epilogue operands.
        aT_sb = const_pool.tile([128, KCH, A_LOC], fp8)
        nc.sync.dma_start(aT_sb[:], aT[:])
        bT_sb = const_pool.tile([128, KCH, B], fp8)
        for c in range(2):
            cs = slice(c * (B // 2), (c + 1) * (B // 2))
            for k in range(KCH):
                nc.sync.dma_start(bT_sb[:, k, cs], bT[k, :, cs])

        eps_sb = const_pool.tile([128, 1], f32)
        nc.vector.memset(eps_sb[:], EPS)
        idx_sb = const_pool.tile([128, NC2], i32)
        nc.sync.dma_start(idx_sb[:], gidx[:])
        ssum_sb = const_pool.tile([128, NC2], f32)
        nc.sync.dma_start(ssum_sb[:], ssum[:])
        bmp_sb = const_pool.tile([128, NC2], f32)
        nc.sync.dma_start(bmp_sb[:], bmp[:])

        # Gram scratch, laid out [p, m, col]: anchor a = 128m + p
        g_t = gdram_pool.tile([128, M_TILES * B], fp8)

        # Gram block: units (column half h, anchor tile m); h-outer so the
        # first 4 units only need the first half of the bT columns.
        # Both copy engines split each unit (halves PSUM release latency).
        for h in range(B // HALF):
            for m in range(M_TILES):
                ps = psum_pool.tile([128, HALF // NBANK, NBANK], f32, tag="ps")
                # k2-outer so PE only waits on bT chunks 2*k2, 2*k2+1
                for k2 in range(KCH // 2):
                    for b in range(HALF // NBANK):
                        nc.tensor.matmul(
                            ps[:, b, :],
                            lhsT=aT_sb[:, 2 * k2:2 * k2 + 2,
                                       m * 128:(m + 1) * 128],
                            rhs=bT_sb[:, 2 * k2:2 * k2 + 2,
                                      h * HALF + b * NBANK:
                                      h * HALF + (b + 1) * NBANK],
                            start=(k2 == 0), stop=(k2 == KCH // 2 - 1),
                            perf_mode=mybir.MatmulPerfMode.DoubleRow,
                        )
                gsb = gsb_pool.tile([128, HALF], fp8, tag="gsb")
                nc.scalar.copy(out=gsb[:, 0:HALF // 2], in_=ps[:, 0:2, :])
                nc.vector.tensor_copy(out=gsb[:, HALF // 2:HALF],
                                      in_=ps[:, 2:4, :])
                nc.sync.dma_start(
                    g_t[:, m * B + h * HALF:m * B + (h + 1) * HALF], gsb[:])

        # Per-triplet dot products: scalar gather from the DRAM Gram block
        g_vals = epi_pool.tile([128, NC2], fp8, tag="gv")
        nc.gpsimd.indirect_dma_start(
            out=g_vals[:],
            out_offset=None,
            in_=g_t[:],
            in_offset=bass.IndirectOffsetOnAxis(ap=idx_sb[:], axis=1),
        )

        # d^2 = ssum - 8*g = -8*(g - ssum/8); host passes ssum/8.
        dif = epi_pool.tile([128, NC2], f32, tag="dif")
        nc.vector.tensor_tensor(
            out=dif[:], in0=g_vals[:], in1=ssum_sb[:],
            op=mybir.AluOpType.subtract)
        dsq = epi_pool.tile([128, NC2], f32, tag="dsq")
        nc.vector.tensor_scalar(
            out=dsq[:], in0=dif[:], scalar1=-8.0, scalar2=0.0,
            op0=mybir.AluOpType.mult, op1=mybir.AluOpType.max)
        dist = epi_pool.tile([128, NC2], f32, tag="dist")
        nc.scalar.activation(
            out=dist[:], in_=dsq[:],
            func=mybir.ActivationFunctionType.Sqrt, bias=eps_sb[:])

        # hinges: pos = relu(d_ap - bm) (DVE), neg = relu(bp - d_an) (Pool)
        hing = epi_pool.tile([128, NC2], f32, tag="hing")
        nc.vector.tensor_tensor(
            out=hing[:, 0:NCOLS], in0=dist[:, 0:NCOLS], in1=bmp_sb[:, 0:NCOLS],
            op=mybir.AluOpType.subtract)
        nc.gpsimd.tensor_tensor(
            out=hing[:, NCOLS:NC2], in0=bmp_sb[:, NCOLS:NC2],
            in1=dist[:, NCOLS:NC2], op=mybir.AluOpType.subtract)
        nc.vector.tensor_scalar_max(hing[:], hing[:], 0.0)
        z = epi_pool.tile([128, NCOLS], f32, tag="z")
        outsb = epi_pool.tile([128, 2], f32, tag="outsb")
        nc.vector.tensor_tensor(
            out=z[:], in0=hing[:, 0:NCOLS], in1=hing[:, NCOLS:NC2],
            op=mybir.AluOpType.add)
        ind = epi_pool.tile([128, NCOLS], f32, tag="ind")
        nc.vector.tensor_scalar(
            out=ind[:], in0=z[:], scalar1=0.0, scalar2=None,
            op0=mybir.AluOpType.is_gt)
        nc.vector.tensor_reduce(
            out=outsb[:, 0:1], in_=z[:], axis=mybir.AxisListType.X,
            op=mybir.AluOpType.add)
        nc.vector.tensor_reduce(
            out=outsb[:, 1:2], in_=ind[:], axis=mybir.AxisListType.X,
            op=mybir.AluOpType.add)
        nc.sync.dma_start(outp[:], outsb[:])

    nc.compile()
    return nc


def _prep_inputs(batch, beta, labels, triplets):
    batch = np.asarray(batch, dtype=np.float32)
    beta = np.asarray(beta, dtype=np.float32)
    labels = np.asarray(labels).astype(np.int64)
    triplets = np.asarray(triplets).astype(np.int64)

    q = (0.5 * batch).astype(FP8NP)          # device rows (scaled by 1/2)
    qf = q.astype(np.float32)
    # effective embedding is 2*q; s = |2q|^2
    s = 4.0 * (qf.astype(np.float64) ** 2).sum(axis=1)
    s = s.astype(np.float32)

    # bT[k, d, j] = q[j, 128k + d]
    bT_all = np.ascontiguousarray(q.T.reshape(KCH, 128, B))

    ia, ip, iN = triplets[:, 0], triplets[:, 1], triplets[:, 2]
    core = ia // A_LOC
    al = ia % A_LOC
    b = beta[labels[ia]].astype(np.float32)

    # Gram scratch layout [p, m, col]: flat = (al%128)*(M_TILES*B) + (al//128)*B + col
    CAP = 128 * NCOLS
    in_maps = []
    for c in range(N_CORES):
        sel = np.nonzero(core == c)[0]
        n = len(sel)
        assert n <= CAP, f"core {c} overflow: {n} > {CAP}"

        def packi(vals, fill):
            arr = np.full(CAP, fill, dtype=np.int64)
            arr[:n] = vals
            return arr.reshape(NCOLS, 128).T.astype(np.int32)

        def packf(vals, fill):
            arr = np.full(CAP, fill, dtype=np.float32)
            arr[:n] = vals
            return arr.reshape(NCOLS, 128).T

        alc = al[sel]
        base = (alc % 128) * (M_TILES * B) + (alc // 128) * B
        gidx = np.concatenate(
            [packi(base + ip[sel], 0), packi(base + iN[sel], 0)], axis=1)
        # device computes d^2 = -8*(g - ssum/8): pass ssum/8
        ssum = np.concatenate(
            [packf((s[ia[sel]] + s[ip[sel]]) / 8.0, 0.0),
             packf((s[ia[sel]] + s[iN[sel]]) / 8.0, 0.0)], axis=1)
        # first half bm (pad +LARGE kills pos), second half bp (pad -LARGE
        # kills neg: device computes relu(bp - d))
        bmp = np.concatenate(
            [packf(b[sel] - MARGIN, LARGE), packf(b[sel] + MARGIN, -LARGE)],
            axis=1)

        aTc = np.ascontiguousarray(
            bT_all[:, :, c * A_LOC:(c + 1) * A_LOC].transpose(1, 0, 2))

        in_maps.append({
            "bT": bT_all,
            "aT": aTc,
            "gidx": np.ascontiguousarray(gidx),
            "ssum": np.ascontiguousarray(ssum),
            "bmp": np.ascontiguousarray(bmp),
        })
    return in_maps


def _finalize(results):
    total = np.float64(0.0)
    cnt = np.float64(0.0)
    for r in results:
        total += r["out"][:, 0].astype(np.float64).sum()
        cnt += r["out"][:, 1].astype(np.float64).sum()
    total = np.float32(total)
    cnt = np.float32(cnt)
    if cnt > 0.0:
        loss = total / max(cnt, np.float32(1.0))
    else:
        loss = total
    return np.float32(loss)


def run_hw(batch, beta, labels, triplets, trace=False, **kw):
    if "nc" not in _CACHE:
        _CACHE["nc"] = _build_nc()
    nc = _CACHE["nc"]
    in_maps = _prep_inputs(batch, beta, labels, triplets)
    res = run_bass_kernel_spmd(nc, in_maps, list(range(N_CORES)), trace=trace, **kw)
    return _finalize(res.results), res


def kernel(batch, beta, labels, triplets):
    loss, _ = run_hw(batch, beta, labels, triplets)
    return loss
